# revision 11
# baseline (speedup 1.0000x reference)
"""Trainium2 Bass kernel for nn_CriticNetwork (gnn_message_passing).

Math: the reference GNN does mean-aggregation over a complete graph with
self-loops, so every node of an env sees the identical per-env mean.  The
whole network collapses to per-env scalars:

  m_b  = mean over the 16 nodes of obs[b]                      [128]
  p_b  = relu(m_b @ W1 + b1) @ W2 + b2                         [64]
  a_b  = p_b . (Wfc @ (Wattn[:64] + Wattn[64:]))               scalar
  w_b  = sigmoid(leaky_relu(a_b, 0.01))                        scalar
  c_b  = p_b . Wv[:64] + bv                                    scalar
  P_bk = pi[b,k] . Wvy ;  Q_bk = (act-pi)[b,k] . Wvy           (Wvy = Wv[64:72])
  xv[b,j] = c_b + (sum_k P_bk + w_b*sum_k Q_bk - w_b*Q_bj)/16
  out x[b*16+d, j] = xv[b,j]   (independent of d)
  out w[b*16+d, j] = w_b

Sharding: data-parallel over envs, 512 envs per core x 8 cores.
"""

import numpy as np
from contextlib import ExitStack

import concourse.bass as bass
import concourse.bacc as bacc
import concourse.tile as tile
from concourse import mybir
from concourse.bass_utils import run_bass_kernel_spmd

B, N, A = 4096, 16, 8
D_IN, H1, DP, DZ = 128, 64, 64, 64
NCORES = 8
BC = B // NCORES          # 512 envs per core
RC = BC * N               # 8192 obs rows per core
G = 4                     # env groups per core
GE = BC // G              # 128 envs per group
CW = 272                  # const tile width

F32 = mybir.dt.float32
ALU = mybir.AluOpType


def _build():
    nc = bacc.Bacc("TRN2", target_bir_lowering=False, debug=False)

    obs = nc.dram_tensor("obs", [RC, D_IN], F32, kind="ExternalInput")
    pa = nc.dram_tensor("pa", [BC, 2 * N * A], F32, kind="ExternalInput")
    cst = nc.dram_tensor("cst", [128, CW], F32, kind="ExternalInput")
    xo = nc.dram_tensor("xo", [RC, N], F32, kind="ExternalOutput")
    wo = nc.dram_tensor("wo", [RC, N], F32, kind="ExternalOutput")

    with ExitStack() as ctx:
        tc = ctx.enter_context(tile.TileContext(nc))
        consts = ctx.enter_context(tc.tile_pool(name="consts", bufs=1))
        obsp = ctx.enter_context(tc.tile_pool(name="obsp", bufs=4))
        pap = ctx.enter_context(tc.tile_pool(name="pap", bufs=4))
        sb = ctx.enter_context(tc.tile_pool(name="sb", bufs=2))
        sbB = ctx.enter_context(tc.tile_pool(name="sbB", bufs=2))
        pmp = ctx.enter_context(tc.tile_pool(name="pmp", bufs=2, space="PSUM"))
        php = ctx.enter_context(tc.tile_pool(name="php", bufs=2, space="PSUM"))
        ppp = ctx.enter_context(tc.tile_pool(name="ppp", bufs=2, space="PSUM"))
        pacp = ctx.enter_context(tc.tile_pool(name="pacp", bufs=1, space="PSUM"))
        ptrp = ctx.enter_context(tc.tile_pool(name="ptrp", bufs=1, space="PSUM"))

        cst_sb = consts.tile([128, CW], F32)
        nc.sync.dma_start(out=cst_sb, in_=cst.ap())
        e8_sb = cst_sb[:, 0:8]            # [128, 8] block-diag 1/16
        wvyb_sb = cst_sb[:, 8:136]        # [128, 128] tiled Wvy
        w1_sb = cst_sb[:, 136:200]        # [128, 64]
        w2_sb = cst_sb[0:64, 200:264]     # [64, 64]
        wac_sb = cst_sb[0:64, 264:266]    # [64, 2] (wa | Wv[:64])
        b1_sb = cst_sb[0:64, 266:267]
        b2_sb = cst_sb[0:64, 267:268]
        id2_sb = cst_sb[0:2, 268:270]     # eye(2)
        bias2_sb = cst_sb[0:2, 270:271]   # [0, bv]
        zeros64 = nc.const_aps.tensor(0.0, (64, GE))

        obs_v = obs.ap().rearrange("(g c p) f -> g p c f", p=128, c=16)
        xo_v = xo.ap().rearrange("(e d) j -> e d j", d=N)
        wo_v = wo.ap().rearrange("(e d) j -> e d j", d=N)

        for g in range(G):
            sl = slice(g * GE, (g + 1) * GE)

            obs_t = obsp.tile([128, 16, 128], F32, name="obs_t")
            nc.sync.dma_start(out=obs_t, in_=obs_v[g])
            pa_t = pap.tile([128, 2 * N * A], F32, name="pa_t")
            nc.sync.dma_start(out=pa_t, in_=pa.ap()[sl, :])

            # meanT[f, e] = mean over nodes of obs for env e (feature-major)
            pm = pmp.tile([128, GE], F32, name="pm")
            for c in range(16):
                nc.tensor.matmul(pm[:, 8 * c:8 * (c + 1)], lhsT=obs_t[:, c, :],
                                 rhs=e8_sb, start=True, stop=True)
            meanT = sb.tile([128, GE], F32, name="meanT")
            nc.vector.tensor_copy(meanT, pm)

            # h = relu(W1.T @ meanT + b1)
            ph = php.tile([64, GE], F32, name="ph")
            nc.tensor.matmul(ph, lhsT=w1_sb, rhs=meanT, start=True, stop=True)
            h_sb = sb.tile([64, GE], F32, name="h_sb")
            nc.vector.scalar_tensor_tensor(out=h_sb, in0=ph, scalar=b1_sb,
                                           in1=zeros64, op0=ALU.add, op1=ALU.max)

            # p = W2.T @ h + b2
            pp = ppp.tile([64, GE], F32, name="pp")
            nc.tensor.matmul(pp, lhsT=w2_sb, rhs=h_sb, start=True, stop=True)
            p_sb = sb.tile([64, GE], F32, name="p_sb")
            nc.vector.tensor_scalar_add(p_sb, pp, b2_sb)

            # row0 = a (attention pre-act), row1 = p . Wv[:64]
            pac = pacp.tile([2, GE], F32, name="pac")
            nc.tensor.matmul(pac, lhsT=wac_sb, rhs=p_sb, start=True, stop=True)

            # row0 stays a; row1 becomes c + bv (per-partition bias [0, bv])
            wc = sb.tile([2, GE], F32, name="wc")
            nc.vector.tensor_scalar_add(wc, pac, bias2_sb)
            lr = sb.tile([1, GE], F32, name="lr")
            nc.vector.scalar_tensor_tensor(out=lr, in0=wc[0:1, :], scalar=0.01,
                                           in1=wc[0:1, :], op0=ALU.mult, op1=ALU.max)
            nc.scalar.activation(out=wc[0:1, :], in_=lr,
                                 func=mybir.ActivationFunctionType.Sigmoid)

            # transpose [2, 128] -> [128, 2]: per-env scalars onto partitions
            ptr = ptrp.tile([128, 2], F32, name="ptr")
            nc.tensor.transpose(ptr, wc, id2_sb)
            wcT = sbB.tile([128, 2], F32, name="wcT")
            nc.vector.tensor_copy(wcT, ptr)
            negw = sbB.tile([128, 1], F32, name="negw")
            nc.vector.tensor_scalar_mul(negw, wcT[:, 0:1], -1.0 / N)

            # P/Q dot products against Wvy, with full-row sums
            tmP = sbB.tile([128, N * A], F32, name="tmP")
            nc.vector.tensor_mul(tmP, pa_t[:, 0:128], wvyb_sb)
            PS = sbB.tile([128, 1], F32, name="PS")
            nc.vector.reduce_sum(out=PS, in_=tmP.rearrange("p (n a) -> p n a", a=A),
                                 axis=mybir.AxisListType.XY)
            qd = sbB.tile([128, N * A], F32, name="qd")
            nc.vector.tensor_sub(qd, pa_t[:, 128:256], pa_t[:, 0:128])
            tmQ = sbB.tile([128, N * A], F32, name="tmQ")
            nc.vector.tensor_mul(tmQ, qd, wvyb_sb)
            Qg = sbB.tile([128, N], F32, name="Qg")
            nc.vector.reduce_sum(out=Qg, in_=tmQ.rearrange("p (n a) -> p n a", a=A),
                                 axis=mybir.AxisListType.X)
            QS = sbB.tile([128, 1], F32, name="QS")
            nc.vector.reduce_sum(out=QS, in_=Qg, axis=mybir.AxisListType.X)

            # xv = c + (PS + w*QS)/16 - (w/16)*Q
            t2 = sbB.tile([128, 1], F32, name="t2")
            nc.vector.scalar_tensor_tensor(out=t2, in0=QS, scalar=wcT[:, 0:1],
                                           in1=PS, op0=ALU.mult, op1=ALU.add)
            base = sbB.tile([128, 1], F32, name="base")
            nc.vector.scalar_tensor_tensor(out=base, in0=t2, scalar=1.0 / N,
                                           in1=wcT[:, 1:2], op0=ALU.mult, op1=ALU.add)
            xv = sbB.tile([128, N], F32, name="xv")
            nc.vector.tensor_scalar(out=xv, in0=Qg, scalar1=negw, scalar2=base,
                                    op0=ALU.mult, op1=ALU.add)

            # outputs: broadcast xv over the 16 node-rows; w over all 256 slots
            # (DMA needs a real contiguous innermost run; stride-0 goes in the
            # middle dim, so w is first widened to 16 real elements on DVE)
            w16 = sbB.tile([128, N], F32, name="w16")
            nc.vector.tensor_copy(w16, wcT[:, 0:1].broadcast_to([128, N]))
            nc.scalar.dma_start(out=xo_v[sl],
                                in_=xv.unsqueeze(1).broadcast_to([128, N, N]))
            nc.scalar.dma_start(out=wo_v[sl],
                                in_=w16.unsqueeze(1).broadcast_to([128, N, N]))

    nc.compile()
    return nc


_NC_CACHE = {}


def _get_nc():
    if "nc" not in _NC_CACHE:
        _NC_CACHE["nc"] = _build()
    return _NC_CACHE["nc"]


def _make_in_maps(inputs):
    obs = np.ascontiguousarray(np.asarray(inputs["obs"], np.float32))
    pol = np.asarray(inputs["policies"], np.float32).reshape(B, N * A)
    act = np.asarray(inputs["actions"], np.float32).reshape(B, N * A)
    W1 = np.asarray(inputs["W1"], np.float32)
    b1 = np.asarray(inputs["b1"], np.float32)
    W2 = np.asarray(inputs["W2"], np.float32)
    b2 = np.asarray(inputs["b2"], np.float32)
    Wfc = np.asarray(inputs["Wfc"], np.float32)
    Wattn = np.asarray(inputs["Wattn"], np.float32)
    Wv = np.asarray(inputs["Wv"], np.float32)
    bv = np.asarray(inputs["bv"], np.float32)

    wa = (Wfc @ (Wattn[:DZ] + Wattn[DZ:]))[:, 0]     # [64]
    wvy = Wv[DP:, 0]                                  # [8]

    cst = np.zeros((128, CW), np.float32)
    r = np.arange(128)
    cst[r, r // 16] = 1.0 / 16.0                      # E8 block-diagonal mean
    cst[:, 8:136] = np.tile(wvy, 16)[None, :]
    cst[:, 136:200] = W1
    cst[0:64, 200:264] = W2
    cst[0:64, 264] = wa
    cst[0:64, 265] = Wv[:DP, 0]
    cst[0:64, 266] = b1
    cst[0:64, 267] = b2
    cst[0:2, 268:270] = np.eye(2, dtype=np.float32)
    cst[0, 270] = 0.0
    cst[1, 270] = bv[0]

    pa = np.ascontiguousarray(np.concatenate([pol, act], axis=1))  # [4096, 256]

    in_maps = []
    for c in range(NCORES):
        in_maps.append({
            "obs": np.ascontiguousarray(obs[c * RC:(c + 1) * RC]),
            "pa": np.ascontiguousarray(pa[c * BC:(c + 1) * BC]),
            "cst": cst,
        })
    return in_maps


# Test-harness knobs (the grader just calls kernel() with defaults).
TRACE = False
TRACE_KWARGS = {}
LAST_RESULT = None


def kernel(**inputs):
    global LAST_RESULT
    nc = _get_nc()
    in_maps = _make_in_maps(inputs)
    res = run_bass_kernel_spmd(nc, in_maps, core_ids=list(range(NCORES)),
                               trace=TRACE, **TRACE_KWARGS)
    LAST_RESULT = res
    x = np.concatenate([r["xo"] for r in res.results], axis=0).reshape(B * N, N, 1)
    w = np.concatenate([r["wo"] for r in res.results], axis=0).reshape(B * N, N, 1)
    return x, w


# revision 13
# speedup vs baseline: 1.1001x; 1.1001x over previous
"""Trainium2 Bass kernel for nn_CriticNetwork (gnn_message_passing).

Math: the reference GNN does mean-aggregation over a complete graph with
self-loops, so every node of an env sees the identical per-env mean.  The
whole network collapses to per-env scalars:

  m_b  = mean over the 16 nodes of obs[b]                      [128]
  p_b  = relu(m_b @ W1 + b1) @ W2 + b2                         [64]
  a_b  = p_b . (Wfc @ (Wattn[:64] + Wattn[64:]))               scalar
  w_b  = sigmoid(leaky_relu(a_b, 0.01))                        scalar
  c_b  = p_b . Wv[:64] + bv                                    scalar
  P_bk = pi[b,k] . Wvy ;  A_bk = act[b,k] . Wvy                (Wvy = Wv[64:72])
  xv[b,j] = c_b + (PS_b + w_b*(AS_b-PS_b) - w_b*(A_bj-P_bj))/16
  out x[b*16+d, j] = xv[b,j]   (independent of d)
  out w[b*16+d, j] = w_b

Sharding: data-parallel over envs, 512 envs per core x 8 cores.

Layouts per core:
  obs group g (128 envs): sbuf [128p=env, 16node*128feat] (contiguous),
    DVE-reduce over node -> meanS [env, feat], PE-transpose -> meanT[feat, env].
  chain per group: W1/16 matmul + relu -> W2 + b2 -> [wa|Wv[:64]] -> a,c rows.
  policies/actions: sbuf [128p, 64row*8act]; all per-row dots batched.
  per-env scalars w,c land in [128p, 4env] via strided sbuf->sbuf DMA.
"""

import numpy as np
from contextlib import ExitStack

import concourse.bass as bass
import concourse.bacc as bacc
import concourse.tile as tile
from concourse import mybir
from concourse.bass_utils import run_bass_kernel_spmd

B, N, A = 4096, 16, 8
D_IN, H1, DP, DZ = 128, 64, 64, 64
NCORES = 8
BC = B // NCORES          # 512 envs per core
RC = BC * N               # 8192 obs rows per core
G = 4                     # env groups per core
GE = BC // G              # 128 envs per group
CW = 272                  # const tile width

F32 = mybir.dt.float32
ALU = mybir.AluOpType
AFT = mybir.ActivationFunctionType


def _build():
    nc = bacc.Bacc("TRN2", target_bir_lowering=False, debug=False)

    obs = nc.dram_tensor("obs", [RC, D_IN], F32, kind="ExternalInput")
    pol = nc.dram_tensor("pol", [128, BC], F32, kind="ExternalInput")
    act = nc.dram_tensor("act", [128, BC], F32, kind="ExternalInput")
    cst = nc.dram_tensor("cst", [128, CW], F32, kind="ExternalInput")
    xo = nc.dram_tensor("xo", [RC, N], F32, kind="ExternalOutput")
    wo = nc.dram_tensor("wo", [RC, N], F32, kind="ExternalOutput")

    with ExitStack() as ctx:
        tc = ctx.enter_context(tile.TileContext(nc))
        consts = ctx.enter_context(tc.tile_pool(name="consts", bufs=1))
        obsp = ctx.enter_context(tc.tile_pool(name="obsp", bufs=2))
        pap = ctx.enter_context(tc.tile_pool(name="pap", bufs=1))
        sb = ctx.enter_context(tc.tile_pool(name="sb", bufs=2))
        sbB = ctx.enter_context(tc.tile_pool(name="sbB", bufs=1))
        pmtp = ctx.enter_context(tc.tile_pool(name="pmtp", bufs=2, space="PSUM"))
        php = ctx.enter_context(tc.tile_pool(name="php", bufs=2, space="PSUM"))
        ppp = ctx.enter_context(tc.tile_pool(name="ppp", bufs=2, space="PSUM"))
        pacp = ctx.enter_context(tc.tile_pool(name="pacp", bufs=2, space="PSUM"))

        cst_sb = consts.tile([128, CW], F32)
        nc.sync.dma_start(out=cst_sb, in_=cst.ap())
        wvy8_sb = cst_sb[:, 0:8]            # Wvy tiled to all partitions
        w1q_sb = cst_sb[:, 8:72]            # W1 / 16
        w2_sb = cst_sb[0:64, 72:136]
        wac_sb = cst_sb[0:64, 136:138]      # [wa | Wv[:64]]
        b1_sb = cst_sb[0:64, 138:139]
        b2_sb = cst_sb[0:64, 139:140]
        bias2_sb = cst_sb[0:2, 140:141]     # [0, bv]
        id128_sb = cst_sb[:, 144:272]       # eye(128)

        # obs rows for env e: 16e..16e+15; group g -> partition p = env-128g
        obs_v = obs.ap().rearrange("(g p nf) f -> g p (nf f)", p=128, nf=16)

        # ---- per-group: mean over nodes, then transpose to [feat, env] ----
        meanT = consts.tile([128, BC], F32)
        for g in range(G):
            obs_t = obsp.tile([128, 16 * 128], F32, name="obs_t")
            nc.sync.dma_start(out=obs_t, in_=obs_v[g])
            meanS = sb.tile([128, 128], F32, name="meanS")
            nc.vector.reduce_sum(out=meanS,
                                 in_=obs_t.rearrange("p (n f) -> p f n", n=16),
                                 axis=mybir.AxisListType.X)
            pmt = pmtp.tile([128, 128], F32, name="pmt")
            nc.tensor.transpose(pmt, meanS[:], id128_sb)
            nc.vector.tensor_copy(meanT[:, g * GE:(g + 1) * GE], pmt)

        pol_sb = pap.tile([128, BC], F32)
        nc.sync.dma_start(out=pol_sb, in_=pol.ap())
        act_sb = pap.tile([128, BC], F32)
        nc.sync.dma_start(out=act_sb, in_=act.ap())

        # ---- per-group chain: mean/16 @ W1 -> relu -> W2+b2 -> [a|c] ----
        wT4 = sbB.tile([128, 4], F32)
        cT4 = sbB.tile([128, 4], F32)
        for g in range(G):
            cols = slice(g * GE, (g + 1) * GE)
            ph = php.tile([64, GE], F32, name="ph")
            nc.tensor.matmul(ph, lhsT=w1q_sb, rhs=meanT[:, cols],
                             start=True, stop=True)
            h_sb = sb.tile([64, GE], F32, name="h_sb")
            nc.scalar.activation(out=h_sb, in_=ph, func=AFT.Relu, bias=b1_sb)
            pp = ppp.tile([64, GE], F32, name="pp")
            nc.tensor.matmul(pp, lhsT=w2_sb, rhs=h_sb, start=True, stop=True)
            p_sb = sb.tile([64, GE], F32, name="p_sb")
            nc.scalar.activation(out=p_sb, in_=pp, func=AFT.Identity, bias=b2_sb)
            pac = pacp.tile([2, GE], F32, name="pac")
            nc.tensor.matmul(pac, lhsT=wac_sb, rhs=p_sb, start=True, stop=True)
            wc = sb.tile([2, GE], F32, name="wc")
            nc.vector.tensor_scalar_add(wc, pac, bias2_sb)
            lr = sb.tile([1, GE], F32, name="lr")
            nc.vector.scalar_tensor_tensor(out=lr, in0=wc[0:1, :], scalar=0.01,
                                           in1=wc[0:1, :], op0=ALU.mult,
                                           op1=ALU.max)
            nc.scalar.activation(out=wc[0:1, :], in_=lr, func=AFT.Sigmoid)
            # scatter per-env scalars onto partitions: wT4[32g+c//4, c%4]
            nc.scalar.dma_start(
                out=wT4[32 * g:32 * (g + 1), :],
                in_=wc[0:1, :].rearrange("o (p i) -> o p i", i=4))
            nc.scalar.dma_start(
                out=cT4[32 * g:32 * (g + 1), :],
                in_=wc[1:2, :].rearrange("o (p i) -> o p i", i=4))

        # ---- batched per-node dots: P = pi.Wvy, A = act.Wvy ----
        wvyb = wvy8_sb.unsqueeze(1).broadcast_to([128, 64, 8])
        tmP = sbB.tile([128, BC], F32)
        nc.vector.tensor_tensor(out=tmP.rearrange("p (r a) -> p r a", a=8),
                                in0=pol_sb.rearrange("p (r a) -> p r a", a=8),
                                in1=wvyb, op=ALU.mult)
        tmA = sbB.tile([128, BC], F32)
        nc.vector.tensor_tensor(out=tmA.rearrange("p (r a) -> p r a", a=8),
                                in0=act_sb.rearrange("p (r a) -> p r a", a=8),
                                in1=wvyb, op=ALU.mult)
        P64 = sbB.tile([128, 64], F32)
        nc.vector.reduce_sum(out=P64, in_=tmP.rearrange("p (r a) -> p r a", a=8),
                             axis=mybir.AxisListType.X)
        A64 = sbB.tile([128, 64], F32)
        nc.vector.reduce_sum(out=A64, in_=tmA.rearrange("p (r a) -> p r a", a=8),
                             axis=mybir.AxisListType.X)
        Q64 = sbB.tile([128, 64], F32)
        nc.vector.tensor_sub(Q64, A64, P64)
        PS4 = sbB.tile([128, 4], F32)
        nc.vector.reduce_sum(out=PS4, in_=P64.rearrange("p (i n) -> p i n", n=16),
                             axis=mybir.AxisListType.X)
        AS4 = sbB.tile([128, 4], F32)
        nc.vector.reduce_sum(out=AS4, in_=A64.rearrange("p (i n) -> p i n", n=16),
                             axis=mybir.AxisListType.X)
        QS4 = sbB.tile([128, 4], F32)
        nc.vector.tensor_sub(QS4, AS4, PS4)

        # ---- combine: xv = c + (PS + w*QS)/16 - (w/16)*Q ----
        negw4 = sbB.tile([128, 4], F32)
        nc.scalar.mul(negw4, wT4[:], -1.0 / N)
        t2 = sbB.tile([128, 4], F32)
        nc.vector.tensor_mul(t2, wT4[:], QS4)
        t3 = sbB.tile([128, 4], F32)
        nc.vector.tensor_add(t3, t2, PS4)
        base4 = sbB.tile([128, 4], F32)
        nc.vector.scalar_tensor_tensor(out=base4, in0=t3, scalar=1.0 / N,
                                       in1=cT4[:], op0=ALU.mult, op1=ALU.add)
        nwq = sbB.tile([128, 64], F32)
        nc.vector.tensor_tensor(out=nwq.rearrange("p (i n) -> p i n", n=16),
                                in0=Q64.rearrange("p (i n) -> p i n", n=16),
                                in1=negw4.unsqueeze(2).broadcast_to([128, 4, 16]),
                                op=ALU.mult)
        xv64 = sbB.tile([128, 64], F32)
        nc.vector.tensor_tensor(out=xv64.rearrange("p (i n) -> p i n", n=16),
                                in0=nwq.rearrange("p (i n) -> p i n", n=16),
                                in1=base4.unsqueeze(2).broadcast_to([128, 4, 16]),
                                op=ALU.add)
        w64 = sbB.tile([128, 64], F32)
        nc.vector.tensor_copy(w64.rearrange("p (i n) -> p i n", n=16),
                              wT4.unsqueeze(2).broadcast_to([128, 4, 16]))

        # ---- outputs: row r = 64p+16i+d gets xv[e=4p+i, :] / w_e ----
        # one DMA per i-block keeps APs within the 3-dim DMA limit
        xo_v = xo.ap().rearrange("(p i d) j -> p i d j", i=4, d=16)
        wo_v = wo.ap().rearrange("(p i d) j -> p i d j", i=4, d=16)
        for i in range(4):
            nc.scalar.dma_start(
                out=wo_v[:, i],
                in_=w64[:, 16 * i:16 * (i + 1)].unsqueeze(1)
                    .broadcast_to([128, 16, 16]))
            nc.scalar.dma_start(
                out=xo_v[:, i],
                in_=xv64[:, 16 * i:16 * (i + 1)].unsqueeze(1)
                    .broadcast_to([128, 16, 16]))

    nc.compile()
    return nc


_NC_CACHE = {}


def _get_nc():
    if "nc" not in _NC_CACHE:
        _NC_CACHE["nc"] = _build()
    return _NC_CACHE["nc"]


def _make_in_maps(inputs):
    obs = np.ascontiguousarray(np.asarray(inputs["obs"], np.float32))
    pol = np.asarray(inputs["policies"], np.float32)
    act = np.asarray(inputs["actions"], np.float32)
    W1 = np.asarray(inputs["W1"], np.float32)
    b1 = np.asarray(inputs["b1"], np.float32)
    W2 = np.asarray(inputs["W2"], np.float32)
    b2 = np.asarray(inputs["b2"], np.float32)
    Wfc = np.asarray(inputs["Wfc"], np.float32)
    Wattn = np.asarray(inputs["Wattn"], np.float32)
    Wv = np.asarray(inputs["Wv"], np.float32)
    bv = np.asarray(inputs["bv"], np.float32)

    wa = (Wfc @ (Wattn[:DZ] + Wattn[DZ:]))[:, 0]     # [64]
    wvy = Wv[DP:, 0]                                  # [8]

    cst = np.zeros((128, CW), np.float32)
    cst[:, 0:8] = wvy[None, :]
    cst[:, 8:72] = W1 / 16.0
    cst[0:64, 72:136] = W2
    cst[0:64, 136] = wa
    cst[0:64, 137] = Wv[:DP, 0]
    cst[0:64, 138] = b1
    cst[0:64, 139] = b2
    cst[0, 140] = 0.0
    cst[1, 140] = bv[0]
    cst[:, 144:272] = np.eye(128, dtype=np.float32)

    in_maps = []
    for c in range(NCORES):
        in_maps.append({
            "obs": np.ascontiguousarray(obs[c * RC:(c + 1) * RC]),
            "pol": np.ascontiguousarray(
                pol[c * RC:(c + 1) * RC].reshape(128, BC)),
            "act": np.ascontiguousarray(
                act[c * RC:(c + 1) * RC].reshape(128, BC)),
            "cst": cst,
        })
    return in_maps


# Test-harness knobs (the grader just calls kernel() with defaults).
TRACE = False
TRACE_KWARGS = {}
LAST_RESULT = None


def kernel(**inputs):
    global LAST_RESULT
    nc = _get_nc()
    in_maps = _make_in_maps(inputs)
    res = run_bass_kernel_spmd(nc, in_maps, core_ids=list(range(NCORES)),
                               trace=TRACE, **TRACE_KWARGS)
    LAST_RESULT = res
    x = np.concatenate([r["xo"] for r in res.results], axis=0).reshape(B * N, N, 1)
    w = np.concatenate([r["wo"] for r in res.results], axis=0).reshape(B * N, N, 1)
    return x, w
